# revision 1
# baseline (speedup 1.0000x reference)
"""Distributed Trainium2 Bass kernel for GQA attention (nn_Attention_27814208209106).

Sharding: 8 cores = 2 batches x 4 KV-head groups.
  Phase 1: x^T via bf16 DMA-transpose (DRAM bounce), per-core q/k/v
           projections (7 q-heads + 1 kv head) + RoPE.
  Phase 2: causal attention in 512-wide T-blocks (k-stationary orientation,
           exp on ScalarE, denominators via ones-matmul), AllGather of each
           block's qkv^T (bf16) within the 4-core batch group overlapped
           with the next block's compute; wo prefetched during attention.
  Phase 3: o-proj per T-block over this core's 896-column output slice.
Host assembles out[b, :, 896*j:896*(j+1)] from core (b, j).

All matmuls in bf16 with f32 PSUM accumulation.
"""

import math
import numpy as np

import concourse.bass as bass
import concourse.mybir as mybir
import concourse.tile as tile
from concourse import bacc
from concourse.bass_utils import run_bass_kernel_spmd

P = 128
FB = 512  # psum free-dim block (f32 psum bank limit)
THETA = 1000000.0

F32 = mybir.dt.float32
BF16 = mybir.dt.bfloat16


class Cfg:
    def __init__(self, T=1024, EMB=3584, NH=28, KVH=4, HD=128):
        self.T, self.EMB, self.NH, self.KVH, self.HD = T, EMB, NH, KVH, HD
        self.GQ = NH // KVH          # q heads per kv head (7)
        self.HG = self.GQ * HD       # per-core q width (896)
        self.NHD = NH * HD           # full qkv width (3584)
        self.EO = EMB // 4           # o-proj output slice per core (896)
        self.KT = EMB // P           # contraction tiles (28)
        self.TT = T // P             # token tiles (8)
        self.NB = (T + FB - 1) // FB  # 512-blocks of T
        self.scale = HD ** -0.5


def _t_blocks(cfg):
    """[(t0, w)] 512-aligned blocks covering [0, T)."""
    return [(b * FB, min(cfg.T, (b + 1) * FB) - b * FB) for b in range(cfg.NB)]


AB = 256  # attention / AllGather chunk width


def _a_chunks(cfg):
    """[(t0, w)] AB-aligned chunks covering [0, T)."""
    n = (cfg.T + AB - 1) // AB
    return [(c * AB, min(cfg.T, (c + 1) * AB) - c * AB) for c in range(n)]


def build_kernel(cfg: Cfg):
    nc = bacc.Bacc(
        "TRN2",
        target_bir_lowering=False,
        debug=False,
        enable_asserts=False,
        num_devices=8,
    )

    xb = nc.dram_tensor("xb", [cfg.EMB, cfg.T], BF16, kind="ExternalInput").ap()
    wq_s = nc.dram_tensor("wq_s", [cfg.EMB, cfg.HG], BF16, kind="ExternalInput").ap()
    wk_s = nc.dram_tensor("wk_s", [cfg.EMB, cfg.HD], BF16, kind="ExternalInput").ap()
    wv_s = nc.dram_tensor("wv_s", [cfg.EMB, cfg.HD], BF16, kind="ExternalInput").ap()
    wo_s = nc.dram_tensor("wo_s", [cfg.NHD, cfg.EO], BF16, kind="ExternalInput").ap()
    cosT = nc.dram_tensor("cosT", [cfg.HD // 2, cfg.T], F32, kind="ExternalInput").ap()
    sinT = nc.dram_tensor("sinT", [cfg.HD // 2, cfg.T], F32, kind="ExternalInput").ap()
    o_s = nc.dram_tensor("o_s", [cfg.T, cfg.EO], F32, kind="ExternalOutput").ap()

    with tile.TileContext(nc) as tc:
        _body(tc, cfg, xb, wq_s, wk_s, wv_s, wo_s, cosT, sinT, o_s)

    nc.compile()
    return nc


def _body(tc, cfg, xb, wq_s, wk_s, wv_s, wo_s, cosT, sinT, o_s):
    nc = tc.nc
    H2 = cfg.HD // 2
    tblocks = _t_blocks(cfg)

    with (
        tc.tile_pool(name="const", bufs=1) as constp,
        tc.tile_pool(name="qT", bufs=cfg.GQ) as qTp,
        tc.tile_pool(name="kT", bufs=1) as kTp,
        tc.tile_pool(name="vv", bufs=cfg.TT) as vp,
        tc.tile_pool(name="dram", bufs=1, space="DRAM") as dramp,
    ):
        # --- constants ---
        ident = constp.tile([P, P], BF16, name="ident")
        nc.gpsimd.memset(ident, 0.0)
        nc.gpsimd.affine_select(
            out=ident, in_=ident, compare_op=mybir.AluOpType.not_equal,
            fill=1.0, base=0, pattern=[[-1, P]], channel_multiplier=1,
        )
        # dmask[s, t] = 1 if s <= t else 0  (valid keys in diag tile)
        dmask = constp.tile([P, P], BF16, name="dmask")
        nc.gpsimd.memset(dmask, 1.0)
        nc.gpsimd.affine_select(
            out=dmask, in_=dmask, compare_op=mybir.AluOpType.is_ge,
            fill=0.0, base=0, pattern=[[1, P]], channel_multiplier=-1,
        )
        ones_bf = constp.tile([P, 1], BF16, name="ones_bf")
        nc.vector.memset(ones_bf, 1.0)
        wrm = constp.tile([P, FB], BF16, name="wrm")
        nc.vector.memset(wrm, 0.0)

        qT = [qTp.tile([P, cfg.T], BF16, name=f"qT{h}", tag="qT") for h in range(cfg.GQ)]
        kT = kTp.tile([P, cfg.T], BF16, name="kT")
        vts = [vp.tile([P, cfg.HD], BF16, name=f"v{i}", tag="v") for i in range(cfg.TT)]

        cc_in = [
            dramp.tile([cfg.HG, w], BF16, name=f"cc_in{b}")
            for b, (t0, w) in enumerate(tblocks)
        ]
        cc_out = [
            dramp.tile([4 * cfg.HG, w], BF16, name=f"cc_out{b}")
            for b, (t0, w) in enumerate(tblocks)
        ]

        # ================= Phase 1: x^T + projections =================
        with (
            tc.tile_pool(name="rope_cs", bufs=1) as csp,
            tc.tile_pool(name="xT", bufs=cfg.KT) as xTp,
            tc.tile_pool(name="wqh", bufs=cfg.KT) as wqhp,
            tc.tile_pool(name="wkvh", bufs=2 * cfg.KT) as wkvhp,
            tc.tile_pool(name="pproj", bufs=4, space="PSUM") as pprojp,
            tc.tile_pool(name="pwarm", bufs=1, space="PSUM") as pwarmp,
            tc.tile_pool(name="pv", bufs=2, space="PSUM") as pvp,
            tc.tile_pool(name="rtmp", bufs=4) as rtp,
        ):
            # PE warmup burst (~4us of dense matmuls while DMA streams in)
            psw = pwarmp.tile([P, FB], F32, name="psw")
            for _ in range(20):
                nc.tensor.matmul(out=psw, lhsT=ident, rhs=wrm, start=True, stop=True)

            cos_sb = csp.tile([H2, cfg.T], F32, name="cos_sb")
            sin_sb = csp.tile([H2, cfg.T], F32, name="sin_sb")
            nc.sync.dma_start(cos_sb, cosT)
            nc.sync.dma_start(sin_sb, sinT)

            # x^T / weights arrive pre-transposed + pre-cast (host marshaling);
            # interleave DMAs so the k/v projections can start immediately
            xTt = [xTp.tile([P, cfg.T], BF16, name=f"xT{k}", tag="xT") for k in range(cfg.KT)]
            wkh, wvh, wqh = [], [], []
            for ke in range(cfg.KT):
                whk = wkvhp.tile([P, cfg.HD], BF16, name=f"wkh{ke}", tag="wkvh")
                nc.sync.dma_start(whk, wk_s[ke * P:(ke + 1) * P, :])
                wkh.append(whk)
                whv = wkvhp.tile([P, cfg.HD], BF16, name=f"wvh{ke}", tag="wkvh")
                nc.sync.dma_start(whv, wv_s[ke * P:(ke + 1) * P, :])
                wvh.append(whv)
                nc.sync.dma_start(xTt[ke], xb[ke * P:(ke + 1) * P, :])
            for ke in range(cfg.KT):
                wh = wqhp.tile([P, cfg.HG], BF16, name=f"wqh{ke}", tag="wqh")
                nc.sync.dma_start(wh, wq_s[ke * P:(ke + 1) * P, :])
                wqh.append(wh)

            def rope_drain(psum, dst, t0, w):
                """dst[:, t0:t0+w] = rope(psum) ; psum [128, w] f32."""
                c = cos_sb[:, t0:t0 + w]
                s = sin_sb[:, t0:t0 + w]
                p1 = psum[0:H2, :]
                p2 = psum[H2:P, :]
                t1 = rtp.tile([H2, FB], F32, name="t1", tag="rt1")[:, :w]
                t2 = rtp.tile([H2, FB], F32, name="t2", tag="rt2")[:, :w]
                nc.vector.tensor_mul(t1, p1, c)
                nc.vector.tensor_mul(t2, p2, s)
                nc.vector.tensor_sub(dst[0:H2, t0:t0 + w], t1, t2)
                nc.vector.tensor_mul(t1, p2, c)
                nc.vector.tensor_mul(t2, p1, s)
                nc.vector.tensor_add(dst[H2:P, t0:t0 + w], t1, t2)

            # k projection + rope (first: attention depends on it)
            psk = [pprojp.tile([P, FB], F32, name=f"psk{i}", tag="pproj")[:, :w]
                   for i, (t0, w) in enumerate(tblocks)]
            for ke in range(cfg.KT):
                for i, (t0, w) in enumerate(tblocks):
                    nc.tensor.matmul(
                        out=psk[i], lhsT=wkh[ke], rhs=xTt[ke][:, t0:t0 + w],
                        start=(ke == 0), stop=(ke == cfg.KT - 1),
                    )
            for i, (t0, w) in enumerate(tblocks):
                rope_drain(psk[i], kT, t0, w)

            # v projection: v[ti] = [128 tok, HD] (token-major, no rope)
            for ti in range(cfg.TT):
                ps = pvp.tile([P, cfg.HD], F32, name="psv", tag="pv")
                for ke in range(cfg.KT):
                    nc.tensor.matmul(
                        out=ps, lhsT=xTt[ke][:, ti * P:(ti + 1) * P], rhs=wvh[ke],
                        start=(ke == 0), stop=(ke == cfg.KT - 1),
                    )
                nc.any.tensor_copy(vts[ti], ps)

            # q projection: stationary wq tile reused across all t-blocks
            for h in range(cfg.GQ):
                pss = [pprojp.tile([P, FB], F32, name=f"psq{i}", tag="pproj")[:, :w]
                       for i, (t0, w) in enumerate(tblocks)]
                for ke in range(cfg.KT):
                    for i, (t0, w) in enumerate(tblocks):
                        nc.tensor.matmul(
                            out=pss[i],
                            lhsT=wqh[ke][:, h * P:(h + 1) * P],
                            rhs=xTt[ke][:, t0:t0 + w],
                            start=(ke == 0), stop=(ke == cfg.KT - 1),
                        )
                for i, (t0, w) in enumerate(tblocks):
                    rope_drain(pss[i], qT[h], t0, w)

        # ============ Phase 2+3: attention, AllGather, o-proj ============
        KO = 4 * cfg.GQ  # 28 contraction tiles of the o-proj
        eblocks = [(e * FB, min(cfg.EO, (e + 1) * FB) - e * FB)
                   for e in range((cfg.EO + FB - 1) // FB)]
        with (
            tc.tile_pool(name="pl", bufs=2, space="PSUM") as plp,
            tc.tile_pool(name="psums", bufs=2, space="PSUM") as psumsp,
            tc.tile_pool(name="pmix", bufs=4, space="PSUM") as pmixp,
            tc.tile_pool(name="pt", bufs=18, space="SBUF") as ptp,
            tc.tile_pool(name="qkvb", bufs=8) as qkvbp,
            tc.tile_pool(name="rec", bufs=6) as recp,
            tc.tile_pool(name="recb", bufs=6) as recbp,
            tc.tile_pool(name="woh", bufs=KO) as wohp,
            tc.tile_pool(name="qkh", bufs=2 * KO) as qkhp,
            tc.tile_pool(name="osb", bufs=3) as osbp,
        ):
            # prefetch wo during attention (pre-cast bf16)
            woh = []
            for kt in range(KO):
                wh = wohp.tile([P, cfg.EO], BF16, name=f"woh{kt}", tag="woh")
                nc.sync.dma_start(wh, wo_s[kt * P:(kt + 1) * P, :])
                woh.append(wh)

            def warm_burst(n):
                pw = pmixp.tile([P, FB], F32, name="pwb", tag="pmix")
                for _ in range(n):
                    nc.tensor.matmul(out=pw, lhsT=ident, rhs=wrm,
                                     start=True, stop=True)

            for tb, (t0b, wb) in enumerate(tblocks):
                si_last = min(cfg.TT - 1, (t0b + wb - 1) // P)
                for h in range(cfg.GQ):
                    pts = []
                    for si in range(si_last + 1):
                        c0 = max(t0b, si * P)
                        cw = t0b + wb - c0
                        pl = plp.tile([P, FB], F32, name="pl", tag="pl")[:, :cw]
                        nc.tensor.matmul(
                            out=pl,
                            lhsT=kT[:, si * P:(si + 1) * P],
                            rhs=qT[h][:, c0:c0 + cw],
                            start=True, stop=True,
                        )
                        pt = ptp.tile([P, FB], BF16, name="pt", tag="pt")[:, :cw]
                        nc.scalar.activation(
                            pt, pl, mybir.ActivationFunctionType.Exp,
                            scale=cfg.scale,
                        )
                        if si * P >= t0b:
                            # diagonal tile: mask invalid (s > t) entries
                            nc.vector.tensor_mul(pt[:, 0:P], pt[:, 0:P], dmask)
                        pts.append((pt, c0, cw))

                    # denominators via ones-matmul over S
                    sp = psumsp.tile([1, FB], F32, name="sums", tag="sums")[:, :wb]
                    for si, (pt, c0, cw) in enumerate(pts):
                        nc.tensor.matmul(
                            out=sp[:, c0 - t0b:c0 - t0b + cw],
                            lhsT=ones_bf, rhs=pt,
                            start=(si == 0), stop=(si == si_last),
                        )
                    rec = recp.tile([1, FB], F32, name="rec", tag="rec")[:, :wb]
                    nc.vector.reciprocal(out=rec, in_=sp)
                    recb = recbp.tile([P, FB], F32, name="recb", tag="recb")[:, :wb]
                    nc.gpsimd.partition_broadcast(recb, rec)

                    # attn @ V (v stationary) + normalize
                    pav = pmixp.tile([P, FB], F32, name="pav", tag="pmix")[:, :wb]
                    for si, (pt, c0, cw) in enumerate(pts):
                        nc.tensor.matmul(
                            out=pav[:, c0 - t0b:c0 - t0b + cw],
                            lhsT=vts[si], rhs=pt,
                            start=(si == 0), stop=(si == si_last),
                        )
                    qkvb = qkvbp.tile([P, FB], BF16, name="qkvb", tag="qkvb")[:, :wb]
                    nc.vector.tensor_mul(qkvb, pav, recb)
                    nc.sync.dma_start(cc_in[tb][h * P:(h + 1) * P, :], qkvb)

                nc.gpsimd.collective_compute(
                    "AllGather",
                    mybir.AluOpType.bypass,
                    replica_groups=[[0, 1, 2, 3], [4, 5, 6, 7]],
                    ins=[cc_in[tb].opt()],
                    outs=[cc_out[tb].opt()],
                )
                warm_burst(12)

            # o-proj per block (block 0 overlaps block 1's AllGather)
            for tb, (t0b, wb) in enumerate(tblocks):
                warm_burst(10)
                qkh = []
                for kt in range(KO):
                    q = qkhp.tile([P, FB], BF16, name=f"qkh{kt}_{tb}", tag="qkh")[:, :wb]
                    nc.sync.dma_start(q, cc_out[tb][kt * P:(kt + 1) * P, :])
                    qkh.append(q)
                for ti in range(wb // P):
                    osb = osbp.tile([P, cfg.EO], F32, name="osb", tag="osb")
                    pos = [
                        pmixp.tile([P, FB], F32, name=f"po{eb}", tag="pmix")[:, :ew]
                        for eb, (e0, ew) in enumerate(eblocks)
                    ]
                    for kt in range(KO):
                        for eb, (e0, ew) in enumerate(eblocks):
                            nc.tensor.matmul(
                                out=pos[eb],
                                lhsT=qkh[kt][:, ti * P:(ti + 1) * P],
                                rhs=woh[kt][:, e0:e0 + ew],
                                start=(kt == 0), stop=(kt == KO - 1),
                            )
                    for eb, (e0, ew) in enumerate(eblocks):
                        nc.any.tensor_copy(osb[:, e0:e0 + ew], pos[eb])
                    nc.sync.dma_start(o_s[t0b + ti * P:t0b + (ti + 1) * P, :], osb)


# ======================= host side =======================

_NC_CACHE = {}


def _get_nc(cfg_key=None):
    if cfg_key not in _NC_CACHE:
        _NC_CACHE[cfg_key] = build_kernel(Cfg())
    return _NC_CACHE[cfg_key]


def _rope_tables(segment_ids, cur_ind, T, HD):
    valid = (np.asarray(segment_ids) != 0)
    pos = np.cumsum(valid, axis=-1) - 1 + int(cur_ind)  # [B, T]
    frac = 2.0 * np.arange(HD // 2, dtype=np.float64) / HD
    timescale = THETA ** frac
    ang = pos[..., None].astype(np.float64) / timescale  # [B, T, HD/2]
    cosT = np.transpose(np.cos(ang), (0, 2, 1)).astype(np.float32)  # [B, HD/2, T]
    sinT = np.transpose(np.sin(ang), (0, 2, 1)).astype(np.float32)
    return cosT, sinT


def prepare_in_maps(inputs, cfg=None):
    import ml_dtypes
    bf16 = ml_dtypes.bfloat16
    cfg = cfg or Cfg()
    x = np.asarray(inputs["x"], dtype=np.float32)
    wq = np.asarray(inputs["wq"], dtype=np.float32).astype(bf16)
    wk = np.asarray(inputs["wk"], dtype=np.float32).astype(bf16)
    wv = np.asarray(inputs["wv"], dtype=np.float32).astype(bf16)
    wo = np.asarray(inputs["wo"], dtype=np.float32).astype(bf16)
    seg = np.asarray(inputs["segment_ids"])
    cur = int(np.asarray(inputs["cur_ind"]))

    B, T, EMB = x.shape
    assert (B, T, EMB) == (2, cfg.T, cfg.EMB)
    HG = cfg.HG
    cosT, sinT = _rope_tables(seg, cur, T, cfg.HD)
    xT = np.ascontiguousarray(np.transpose(x, (0, 2, 1))).astype(bf16)  # [B, EMB, T]

    in_maps = []
    for c in range(8):
        b, j = c // 4, c % 4
        in_maps.append({
            "xb": xT[b],
            "wq_s": np.ascontiguousarray(wq[:, j * HG:(j + 1) * HG]),
            "wk_s": np.ascontiguousarray(wk[:, j * cfg.HD:(j + 1) * cfg.HD]),
            "wv_s": np.ascontiguousarray(wv[:, j * cfg.HD:(j + 1) * cfg.HD]),
            "wo_s": np.ascontiguousarray(wo[:, j * cfg.EO:(j + 1) * cfg.EO]),
            "cosT": np.ascontiguousarray(cosT[b]),
            "sinT": np.ascontiguousarray(sinT[b]),
        })
    return in_maps


def assemble_out(results, cfg=None):
    cfg = cfg or Cfg()
    out = np.empty((2, cfg.T, cfg.EMB), np.float32)
    for c in range(8):
        b, j = c // 4, c % 4
        out[b, :, j * cfg.EO:(j + 1) * cfg.EO] = results[c]["o_s"]
    return out


def kernel(**inputs):
    cfg = Cfg()
    in_maps = prepare_in_maps(inputs, cfg)
    nc = _get_nc()
    res = run_bass_kernel_spmd(nc, in_maps, core_ids=list(range(8)))
    return assemble_out(res.results, cfg)



# revision 11
# speedup vs baseline: 1.4330x; 1.4330x over previous
"""Distributed Trainium2 Bass kernel for GQA attention (nn_Attention_27814208209106).

Sharding: 8 cores = 2 batches x 4 KV-head groups (7 q-heads + 1 kv head each).
v2: 4x256-token block pipeline so the four small AllGathers (1.83MB out)
overlap compute; reciprocal broadcast + causal diag mask moved onto the PE
(frees the GpSimd queue, whose collective-completion wait otherwise stalls
later blocks); o-proj in emb-partitioned layout.

Per block b: k/v-proj -> q-proj -> per-head attention (k-stationary, exp on
ScalarE, denominators via ones-matmul) -> qkv^T staged to DRAM -> AllGather
within the 4-core batch group -> o-proj of this core's 896-col slice.
Host assembles out[b, :, 896*j:896*(j+1)] from core (b, j).

All matmuls bf16 with f32 PSUM accumulation. PSUM banks are packed with
region-disjoint accumulation chains (8-bank budget).
"""

import numpy as np

import concourse.bass as bass
import concourse.mybir as mybir
import concourse.tile as tile
from concourse import bacc
from concourse.bass_utils import run_bass_kernel_spmd

P = 128
BW = 256           # token block width
NB = 4             # number of token blocks
THETA = 1000000.0
NEG = -30000.0

F32 = mybir.dt.float32
BF16 = mybir.dt.bfloat16


class Cfg:
    def __init__(self, T=1024, EMB=3584, NH=28, KVH=4, HD=128):
        self.T, self.EMB, self.NH, self.KVH, self.HD = T, EMB, NH, KVH, HD
        self.GQ = NH // KVH          # q heads per kv head (7)
        self.HG = self.GQ * HD       # per-core q width (896)
        self.NHD = NH * HD           # full qkv width (3584)
        self.EO = EMB // 4           # o-proj output slice per core (896)
        self.KT = EMB // P           # contraction tiles (28)
        self.ET = self.EO // P       # o-proj emb tiles (7)
        self.scale = HD ** -0.5


def build_kernel(cfg: Cfg):
    nc = bacc.Bacc(
        "TRN2",
        target_bir_lowering=False,
        debug=False,
        enable_asserts=False,
        num_devices=8,
    )

    xb = nc.dram_tensor("xb", [cfg.EMB, cfg.T], BF16, kind="ExternalInput").ap()
    wq_s = nc.dram_tensor("wq_s", [cfg.EMB, cfg.HG], BF16, kind="ExternalInput").ap()
    wk_s = nc.dram_tensor("wk_s", [cfg.EMB, cfg.HD], BF16, kind="ExternalInput").ap()
    wv_s = nc.dram_tensor("wv_s", [cfg.EMB, cfg.HD], BF16, kind="ExternalInput").ap()
    wo_s = nc.dram_tensor("wo_s", [cfg.NHD, cfg.EO], BF16, kind="ExternalInput").ap()
    cosT = nc.dram_tensor("cosT", [cfg.HD // 2, cfg.T], F32, kind="ExternalInput").ap()
    sinT = nc.dram_tensor("sinT", [cfg.HD // 2, cfg.T], F32, kind="ExternalInput").ap()
    oT_s = nc.dram_tensor("oT_s", [cfg.EO, cfg.T], F32, kind="ExternalOutput").ap()

    with tile.TileContext(nc) as tc:
        _body(tc, cfg, xb, wq_s, wk_s, wv_s, wo_s, cosT, sinT, oT_s)

    nc.compile()
    return nc


def _body(tc, cfg, xb, wq_s, wk_s, wv_s, wo_s, cosT, sinT, oT_s):
    nc = tc.nc
    H2 = cfg.HD // 2

    from contextlib import ExitStack
    with ExitStack() as _st:
        ec = _st.enter_context
        constp = ec(tc.tile_pool(name="const", bufs=1))
        csp = ec(tc.tile_pool(name="cs", bufs=1))
        xTp = ec(tc.tile_pool(name="xT", bufs=2 * cfg.KT))
        wqhp = ec(tc.tile_pool(name="wqh", bufs=cfg.KT))
        wkvhp = ec(tc.tile_pool(name="wkvh", bufs=2 * cfg.KT))
        wohp = ec(tc.tile_pool(name="woh", bufs=cfg.KT))
        kTp = ec(tc.tile_pool(name="kT", bufs=1))
        vp = ec(tc.tile_pool(name="vv", bufs=2 * NB))
        qTp = ec(tc.tile_pool(name="qT", bufs=2 * cfg.GQ))
        ptp = ec(tc.tile_pool(name="pt", bufs=16))
        recp = ec(tc.tile_pool(name="rec", bufs=4))
        qkvbp = ec(tc.tile_pool(name="qkvb", bufs=4))
        qkhp = ec(tc.tile_pool(name="qkh", bufs=42))
        osbp = ec(tc.tile_pool(name="osb", bufs=3))
        rtp = ec(tc.tile_pool(name="rtmp", bufs=4))
        dramp = ec(tc.tile_pool(name="dram", bufs=1, space="DRAM"))
        # PSUM: 8 banks x 2KB/partition. Logical accumulators are packed
        # into shared banks with region-disjoint chains.
        pkvp = ec(tc.tile_pool(name="pkv", bufs=1, space="PSUM"))  # 1 bank
        pqp = ec(tc.tile_pool(name="pq", bufs=2, space="PSUM"))    # 2 banks
        plp = ec(tc.tile_pool(name="pl", bufs=1, space="PSUM"))    # 1 bank
        sprp = ec(tc.tile_pool(name="spr", bufs=1, space="PSUM"))  # 1 bank
        pavp = ec(tc.tile_pool(name="pav", bufs=1, space="PSUM"))  # 1 bank
        pop = ec(tc.tile_pool(name="po", bufs=2, space="PSUM"))    # 2 banks
        # ---- constants ----
        ident = constp.tile([P, P], BF16, name="ident")
        nc.gpsimd.memset(ident, 0.0)
        nc.gpsimd.affine_select(
            out=ident, in_=ident, compare_op=mybir.AluOpType.not_equal,
            fill=1.0, base=0, pattern=[[-1, P]], channel_multiplier=1,
        )
        # negd = diag(NEG)
        negd = constp.tile([P, P], BF16, name="negd")
        nc.gpsimd.memset(negd, 0.0)
        nc.gpsimd.affine_select(
            out=negd, in_=negd, compare_op=mybir.AluOpType.not_equal,
            fill=NEG, base=0, pattern=[[-1, P]], channel_multiplier=1,
        )
        # rlow[s, q] = 1 iff s > q  (strict lower triangle)
        rlow = constp.tile([P, P], BF16, name="rlow")
        nc.gpsimd.memset(rlow, 0.0)
        nc.gpsimd.affine_select(
            out=rlow, in_=rlow, compare_op=mybir.AluOpType.is_ge,
            fill=1.0, base=0, pattern=[[1, P]], channel_multiplier=-1,
        )
        ones_bf = constp.tile([P, 1], BF16, name="ones_bf")
        nc.vector.memset(ones_bf, 1.0)
        ones1 = constp.tile([1, P], BF16, name="ones1")
        nc.vector.memset(ones1, 1.0)
        wrm = constp.tile([P, BW], BF16, name="wrm")
        nc.vector.memset(wrm, 0.0)

        def warm_burst(n):
            pw = pop.tile([P, BW], F32, name="pwb", tag="po")
            for _ in range(n):
                nc.tensor.matmul(out=pw, lhsT=ident, rhs=wrm, start=True, stop=True)

        # PE warmup while first DMAs stream in
        warm_burst(24)

        # ---- DMA staging ----
        cos_sb = csp.tile([H2, cfg.T], F32, name="cos_sb")
        sin_sb = csp.tile([H2, cfg.T], F32, name="sin_sb")
        wkh, wvh = [], []
        for ke in range(cfg.KT):
            whk = wkvhp.tile([P, cfg.HD], BF16, name=f"wkh{ke}", tag="wkvh")
            nc.sync.dma_start(whk, wk_s[ke * P:(ke + 1) * P, :])
            wkh.append(whk)
            whv = wkvhp.tile([P, cfg.HD], BF16, name=f"wvh{ke}", tag="wkvh")
            nc.sync.dma_start(whv, wv_s[ke * P:(ke + 1) * P, :])
            wvh.append(whv)
        nc.sync.dma_start(cos_sb, cosT)
        nc.sync.dma_start(sin_sb, sinT)

        # x block 0 first, then wq, then x b1..b3, then wo
        xt = {}  # (ke, b) -> tile
        for ke in range(cfg.KT):
            t = xTp.tile([P, BW], BF16, name=f"x{ke}_0", tag="xT")
            nc.sync.dma_start(t, xb[ke * P:(ke + 1) * P, 0:BW])
            xt[(ke, 0)] = t
        wqh = []
        for ke in range(cfg.KT):
            wh = wqhp.tile([P, cfg.HG], BF16, name=f"wqh{ke}", tag="wqh")
            nc.sync.dma_start(wh, wq_s[ke * P:(ke + 1) * P, :])
            wqh.append(wh)
        for b in range(1, NB):
            for ke in range(cfg.KT):
                t = xTp.tile([P, BW], BF16, name=f"x{ke}_{b}", tag="xT")
                nc.sync.dma_start(t, xb[ke * P:(ke + 1) * P, b * BW:(b + 1) * BW])
                xt[(ke, b)] = t
        woh = []
        for kt in range(cfg.KT):
            wh = wohp.tile([P, cfg.EO], BF16, name=f"woh{kt}", tag="woh")
            nc.sync.dma_start(wh, wo_s[kt * P:(kt + 1) * P, :])
            woh.append(wh)

        kT = kTp.tile([P, cfg.T], BF16, name="kT")
        vts = [vp.tile([P, cfg.HD], BF16, name=f"v{i}", tag="v")
               for i in range(2 * NB)]

        cc_in = [dramp.tile([cfg.HG, BW], BF16, name=f"cc_in{b}") for b in range(NB)]
        cc_out = [
            dramp.tile([4 * cfg.HG, BW], BF16, name=f"cc_out{b}")
            for b in range(NB)
        ]

        def rope_drain(psum, dst, t0, w, ang0=None):
            """dst[:, t0:t0+w] = rope(psum); psum [128, w] f32, dst bf16.
            ang0: global token offset for the cos/sin tables (default t0)."""
            ang0 = t0 if ang0 is None else ang0
            c = cos_sb[:, ang0:ang0 + w]
            s = sin_sb[:, ang0:ang0 + w]
            p1 = psum[0:H2, :]
            p2 = psum[H2:P, :]
            t1 = rtp.tile([H2, BW], F32, name="t1", tag="rt1")[:, :w]
            t2 = rtp.tile([H2, BW], F32, name="t2", tag="rt2")[:, :w]
            nc.vector.tensor_mul(t1, p1, c)
            nc.vector.tensor_mul(t2, p2, s)
            nc.vector.tensor_sub(dst[0:H2, t0:t0 + w], t1, t2)
            nc.vector.tensor_mul(t1, p2, c)
            nc.vector.tensor_mul(t2, p1, s)
            nc.vector.tensor_add(dst[H2:P, t0:t0 + w], t1, t2)

        qT = {}  # (h, b) -> tile

        def kv_proj(b):
            t0 = b * BW
            # one bank: k in cols [0:256), v sub-tiles in [256:384) / [384:512)
            bank = pkvp.tile([P, 512], F32, name="pkv", tag="pkv")
            psk = bank[:, 0:BW]
            for ke in range(cfg.KT):
                nc.tensor.matmul(out=psk, lhsT=wkh[ke], rhs=xt[(ke, b)],
                                 start=(ke == 0), stop=(ke == cfg.KT - 1))
            rope_drain(psk, kT, t0, BW)
            for sub in range(2):
                pv = bank[:, BW + sub * P:BW + (sub + 1) * P]
                for ke in range(cfg.KT):
                    nc.tensor.matmul(
                        out=pv, lhsT=xt[(ke, b)][:, sub * P:(sub + 1) * P],
                        rhs=wvh[ke],
                        start=(ke == 0), stop=(ke == cfg.KT - 1))
                nc.any.tensor_copy(vts[2 * b + sub], pv)

        def q_proj_heads(b, heads, ke_outer):
            if ke_outer:
                # one bank per head (<=2 heads) so the two interleaved
                # accumulation chains never share a PSUM bank
                assert len(heads) <= 2
                slot = {h: pqp.tile([P, 512], F32, name=f"pqo{h}",
                                    tag="pq")[:, 0:BW] for h in heads}
                for ke in range(cfg.KT):
                    for h in heads:
                        nc.tensor.matmul(
                            out=slot[h], lhsT=wqh[ke][:, h * P:(h + 1) * P],
                            rhs=xt[(ke, b)],
                            start=(ke == 0), stop=(ke == cfg.KT - 1))
                for h in heads:
                    q = qTp.tile([P, BW], BF16, name=f"qT{h}_{b}", tag="qT")
                    rope_drain(slot[h], q, 0, BW, ang0=b * BW)
                    qT[(h, b)] = q
            else:
                # sequential heads: pack pairs into one bank (chains are
                # temporally disjoint; rope of h overlaps proj of h+1)
                bank = None
                for i, h in enumerate(heads):
                    if i % 2 == 0:
                        bank = pqp.tile([P, 512], F32, name=f"pq{h}", tag="pq")
                    ps = bank[:, (i % 2) * BW:(i % 2 + 1) * BW]
                    for ke in range(cfg.KT):
                        nc.tensor.matmul(
                            out=ps, lhsT=wqh[ke][:, h * P:(h + 1) * P],
                            rhs=xt[(ke, b)],
                            start=(ke == 0), stop=(ke == cfg.KT - 1))
                    q = qTp.tile([P, BW], BF16, name=f"qT{h}_{b}", tag="qT")
                    rope_drain(ps, q, 0, BW, ang0=b * BW)
                    qT[(h, b)] = q

        def attn_head(b, h):
            """Attention for head h, token block b -> qkvb -> cc_in[b]."""
            nsi = 2 * (b + 1)
            plbank = plp.tile([P, 512], F32, name="plb", tag="pl")
            pts = []
            for si in range(nsi):
                c0 = 0 if si <= 2 * b else P
                cw = BW - c0
                pl = plbank[:, (si % 2) * BW:(si % 2) * BW + cw]
                diag = si >= 2 * b
                nc.tensor.matmul(
                    out=pl,
                    lhsT=kT[:, si * P:(si + 1) * P],
                    rhs=qT[(h, b)][:, c0:BW],
                    start=True, stop=not diag,
                )
                if diag:
                    # add NEG to masked (s_local > q_local) entries of the
                    # first 128 columns of this si's valid q range
                    nc.tensor.matmul(
                        out=pl[:, 0:P], lhsT=negd, rhs=rlow,
                        start=False, stop=True,
                    )
                pt = ptp.tile([P, BW], BF16, name="pt", tag="pt")[:, :cw]
                nc.scalar.activation(
                    pt, pl, mybir.ActivationFunctionType.Exp, scale=cfg.scale,
                )
                pts.append((pt, c0, cw))

            # shared bank: recb at [0:256), denominators sp at [0:1, 256:512)
            sprbank = sprp.tile([P, 512], F32, name="spr", tag="spr")
            sp = sprbank[0:1, BW:2 * BW]
            for si, (pt, c0, cw) in enumerate(pts):
                nc.tensor.matmul(
                    out=sp[:, c0:c0 + cw], lhsT=ones_bf, rhs=pt,
                    start=(si == 0), stop=(si == nsi - 1),
                )
            rec_bf = recp.tile([1, BW], BF16, name="rec", tag="rec")
            with nc.allow_low_precision("softmax reciprocal feeds bf16 matmul"):
                nc.vector.reciprocal(out=rec_bf, in_=sp)
            recb_ps = sprbank[:, 0:BW]
            nc.tensor.matmul(out=recb_ps, lhsT=ones1, rhs=rec_bf,
                             start=True, stop=True)
            # TensorTensor may read only one PSUM operand: drain recb to SBUF
            recb = recp.tile([P, BW], BF16, name="recb", tag="recb")
            nc.vector.tensor_copy(recb, recb_ps)

            pav = pavp.tile([P, BW], F32, name="pav", tag="pav")
            for si, (pt, c0, cw) in enumerate(pts):
                nc.tensor.matmul(
                    out=pav[:, c0:c0 + cw], lhsT=vts[si], rhs=pt,
                    start=(si == 0), stop=(si == nsi - 1),
                )
            qkvb = qkvbp.tile([P, BW], BF16, name="qkvb", tag="qkvb")
            nc.vector.tensor_mul(qkvb, pav, recb)
            nc.sync.dma_start(cc_in[b][h * P:(h + 1) * P, :], qkvb)

        def allgather(b):
            nc.gpsimd.collective_compute(
                "AllGather",
                mybir.AluOpType.bypass,
                replica_groups=[[0, 1, 2, 3], [4, 5, 6, 7]],
                ins=[cc_in[b].opt()],
                outs=[cc_out[b].opt()],
            )

        def oproj(b):
            qkh = []
            for kt in range(cfg.NH):
                q = qkhp.tile([P, BW], BF16, name=f"qkh{kt}_{b}", tag="qkh")
                nc.sync.dma_start(q, cc_out[b][kt * P:(kt + 1) * P, :])
                qkh.append(q)
            for e in range(cfg.ET):
                po = pop.tile([P, BW], F32, name="po", tag="po")
                for kt in range(cfg.NH):
                    nc.tensor.matmul(
                        out=po, lhsT=woh[kt][:, e * P:(e + 1) * P], rhs=qkh[kt],
                        start=(kt == 0), stop=(kt == cfg.NH - 1),
                    )
                osb = osbp.tile([P, BW], F32, name="osb", tag="osb")
                nc.any.tensor_copy(osb, po)
                nc.sync.dma_start(
                    oT_s[e * P:(e + 1) * P, b * BW:(b + 1) * BW], osb)

        # ================= pipeline =================
        kv_proj(0)
        q_proj_heads(0, [0, 1], ke_outer=True)
        attn_head(0, 0)
        attn_head(0, 1)
        q_proj_heads(0, [2, 3, 4, 5, 6], ke_outer=False)
        for h in range(2, cfg.GQ):
            attn_head(0, h)
        allgather(0)

        for b in range(1, NB):
            kv_proj(b)
            q_proj_heads(b, list(range(cfg.GQ)), ke_outer=False)
            for h in range(cfg.GQ):
                attn_head(b, h)
            allgather(b)
            oproj(b - 1)
        oproj(NB - 1)


# ======================= host side =======================

_NC_CACHE = {}


def _get_nc(cfg_key=None):
    if cfg_key not in _NC_CACHE:
        _NC_CACHE[cfg_key] = build_kernel(Cfg())
    return _NC_CACHE[cfg_key]


def _rope_tables(segment_ids, cur_ind, T, HD):
    valid = (np.asarray(segment_ids) != 0)
    pos = np.cumsum(valid, axis=-1) - 1 + int(cur_ind)  # [B, T]
    frac = 2.0 * np.arange(HD // 2, dtype=np.float64) / HD
    timescale = THETA ** frac
    ang = pos[..., None].astype(np.float64) / timescale  # [B, T, HD/2]
    cosT = np.transpose(np.cos(ang), (0, 2, 1)).astype(np.float32)  # [B, HD/2, T]
    sinT = np.transpose(np.sin(ang), (0, 2, 1)).astype(np.float32)
    return cosT, sinT


def prepare_in_maps(inputs, cfg=None):
    import ml_dtypes
    bf16 = ml_dtypes.bfloat16
    cfg = cfg or Cfg()
    x = np.asarray(inputs["x"], dtype=np.float32)
    wq = np.asarray(inputs["wq"], dtype=np.float32).astype(bf16)
    wk = np.asarray(inputs["wk"], dtype=np.float32).astype(bf16)
    wv = np.asarray(inputs["wv"], dtype=np.float32).astype(bf16)
    wo = np.asarray(inputs["wo"], dtype=np.float32).astype(bf16)
    seg = np.asarray(inputs["segment_ids"])
    cur = int(np.asarray(inputs["cur_ind"]))

    B, T, EMB = x.shape
    assert (B, T, EMB) == (2, cfg.T, cfg.EMB)
    HG = cfg.HG
    cosT, sinT = _rope_tables(seg, cur, T, cfg.HD)
    xT = np.ascontiguousarray(np.transpose(x, (0, 2, 1))).astype(bf16)  # [B, EMB, T]

    in_maps = []
    for c in range(8):
        b, j = c // 4, c % 4
        in_maps.append({
            "xb": xT[b],
            "wq_s": np.ascontiguousarray(wq[:, j * HG:(j + 1) * HG]),
            "wk_s": np.ascontiguousarray(wk[:, j * cfg.HD:(j + 1) * cfg.HD]),
            "wv_s": np.ascontiguousarray(wv[:, j * cfg.HD:(j + 1) * cfg.HD]),
            "wo_s": np.ascontiguousarray(wo[:, j * cfg.EO:(j + 1) * cfg.EO]),
            "cosT": np.ascontiguousarray(cosT[b]),
            "sinT": np.ascontiguousarray(sinT[b]),
        })
    return in_maps


def assemble_out(results, cfg=None):
    cfg = cfg or Cfg()
    out = np.empty((2, cfg.T, cfg.EMB), np.float32)
    for c in range(8):
        b, j = c // 4, c % 4
        out[b, :, j * cfg.EO:(j + 1) * cfg.EO] = results[c]["oT_s"].T
    return out


def kernel(**inputs):
    cfg = Cfg()
    in_maps = prepare_in_maps(inputs, cfg)
    nc = _get_nc()
    res = run_bass_kernel_spmd(nc, in_maps, core_ids=list(range(8)))
    return assemble_out(res.results, cfg)


# revision 16
# speedup vs baseline: 1.7340x; 1.2100x over previous
"""Distributed Trainium2 Bass kernel for GQA attention (nn_Attention_27814208209106).

Sharding: 8 cores = 2 batches x 4 KV-head groups (7 q-heads + 1 kv head each).
v3: 4x256-token block pipeline with batched DMAs (host pre-packs every
weight/activation into partition-major monoliths so each logical transfer is
ONE descriptor: ~25 DMA issues total vs ~260 in v2 -- the SP sequencer at
~565ns/issue was gating the input stream). Small AllGathers (1.83MB out, one
per block) overlap compute; reciprocal broadcast + causal diag mask run on
the PE; o-proj in emb-partitioned layout.

All matmuls bf16 with f32 PSUM accumulation. PSUM banks are packed with
region-disjoint accumulation chains (8-bank budget).
"""

import numpy as np

import concourse.bass as bass
import concourse.mybir as mybir
import concourse.tile as tile
from concourse import bacc
from concourse.bass_utils import run_bass_kernel_spmd

P = 128
BW = 256           # token block width
NB = 4             # number of token blocks
THETA = 1000000.0
NEG = -30000.0

F32 = mybir.dt.float32
BF16 = mybir.dt.bfloat16


class Cfg:
    def __init__(self, T=1024, EMB=3584, NH=28, KVH=4, HD=128):
        self.T, self.EMB, self.NH, self.KVH, self.HD = T, EMB, NH, KVH, HD
        self.GQ = NH // KVH          # q heads per kv head (7)
        self.HG = self.GQ * HD       # per-core q width (896)
        self.NHD = NH * HD           # full qkv width (3584)
        self.EO = EMB // 4           # o-proj output slice per core (896)
        self.KT = EMB // P           # contraction tiles (28)
        self.ET = self.EO // P       # o-proj emb tiles (7)
        self.scale = HD ** -0.5


def build_kernel(cfg: Cfg):
    nc = bacc.Bacc(
        "TRN2",
        target_bir_lowering=False,
        debug=False,
        enable_asserts=False,
        num_devices=8,
    )

    KT, HG, EO = cfg.KT, cfg.HG, cfg.EO
    xb2 = nc.dram_tensor("xb2", [NB * P, KT * BW], BF16, kind="ExternalInput").ap()
    wq2 = nc.dram_tensor("wq2", [P, KT * HG], BF16, kind="ExternalInput").ap()
    wkv2 = nc.dram_tensor("wkv2", [P, 2 * KT * cfg.HD], BF16, kind="ExternalInput").ap()
    wo2 = nc.dram_tensor("wo2", [P, KT * EO], BF16, kind="ExternalInput").ap()
    cosT = nc.dram_tensor("cosT", [cfg.HD // 2, cfg.T], BF16, kind="ExternalInput").ap()
    sinT = nc.dram_tensor("sinT", [cfg.HD // 2, cfg.T], BF16, kind="ExternalInput").ap()
    oT_s = nc.dram_tensor("oT_s", [EO, cfg.T], F32, kind="ExternalOutput").ap()

    with tile.TileContext(nc) as tc:
        _body(tc, cfg, xb2, wq2, wkv2, wo2, cosT, sinT, oT_s)

    nc.compile()
    return nc


def _body(tc, cfg, xb2, wq2, wkv2, wo2, cosT, sinT, oT_s):
    nc = tc.nc
    H2 = cfg.HD // 2
    KT, HG, EO, NH, GQ = cfg.KT, cfg.HG, cfg.EO, cfg.NH, cfg.GQ

    from contextlib import ExitStack
    with ExitStack() as _st:
        ec = _st.enter_context
        constp = ec(tc.tile_pool(name="const", bufs=1))
        csp = ec(tc.tile_pool(name="cs", bufs=1))
        xTp = ec(tc.tile_pool(name="xT", bufs=2))
        wqp = ec(tc.tile_pool(name="wq", bufs=1))
        wkvp = ec(tc.tile_pool(name="wkv", bufs=1))
        wop = ec(tc.tile_pool(name="wo", bufs=1))
        kTp = ec(tc.tile_pool(name="kT", bufs=1))
        vp = ec(tc.tile_pool(name="vv", bufs=2 * NB))
        qTp = ec(tc.tile_pool(name="qT", bufs=2 * cfg.GQ))
        ptp = ec(tc.tile_pool(name="pt", bufs=10))
        recp = ec(tc.tile_pool(name="rec", bufs=4))
        qkvbp = ec(tc.tile_pool(name="qkvb", bufs=2))
        qkhp = ec(tc.tile_pool(name="qkh", bufs=1))
        osbp = ec(tc.tile_pool(name="osb", bufs=2))
        rtp = ec(tc.tile_pool(name="rtmp", bufs=2))
        dramp = ec(tc.tile_pool(name="dram", bufs=1, space="DRAM"))
        # PSUM: 8 banks x 2KB/partition, region-packed accumulation chains.
        pkvp = ec(tc.tile_pool(name="pkv", bufs=1, space="PSUM"))  # 1 bank
        pqp = ec(tc.tile_pool(name="pq", bufs=2, space="PSUM"))    # 2 banks
        plp = ec(tc.tile_pool(name="pl", bufs=1, space="PSUM"))    # 1 bank
        sprp = ec(tc.tile_pool(name="spr", bufs=1, space="PSUM"))  # 1 bank
        pavp = ec(tc.tile_pool(name="pav", bufs=1, space="PSUM"))  # 1 bank
        pop = ec(tc.tile_pool(name="po", bufs=2, space="PSUM"))    # 2 banks

        # ---- constants ----
        ident = constp.tile([P, P], BF16, name="ident")
        nc.gpsimd.memset(ident, 0.0)
        nc.gpsimd.affine_select(
            out=ident, in_=ident, compare_op=mybir.AluOpType.not_equal,
            fill=1.0, base=0, pattern=[[-1, P]], channel_multiplier=1,
        )
        # negd = diag(NEG)
        negd = constp.tile([P, P], BF16, name="negd")
        nc.gpsimd.memset(negd, 0.0)
        nc.gpsimd.affine_select(
            out=negd, in_=negd, compare_op=mybir.AluOpType.not_equal,
            fill=NEG, base=0, pattern=[[-1, P]], channel_multiplier=1,
        )
        # rlow[s, q] = 1 iff s > q  (strict lower triangle)
        rlow = constp.tile([P, P], BF16, name="rlow")
        nc.gpsimd.memset(rlow, 0.0)
        nc.gpsimd.affine_select(
            out=rlow, in_=rlow, compare_op=mybir.AluOpType.is_ge,
            fill=1.0, base=0, pattern=[[1, P]], channel_multiplier=-1,
        )
        ones_bf = constp.tile([P, 1], BF16, name="ones_bf")
        nc.vector.memset(ones_bf, 1.0)
        ones1 = constp.tile([1, P], BF16, name="ones1")
        nc.vector.memset(ones1, 1.0)
        wrm = constp.tile([P, BW], BF16, name="wrm")
        nc.vector.memset(wrm, 0.0)

        def warm_burst(n):
            pw = pop.tile([P, BW], F32, name="pwb", tag="po")
            for _ in range(n):
                nc.tensor.matmul(out=pw, lhsT=ident, rhs=wrm, start=True, stop=True)

        # PE warmup while first DMAs stream in
        warm_burst(40)

        # ---- batched DMA staging ----
        cos_sb = csp.tile([H2, cfg.T], BF16, name="cos_sb")
        sin_sb = csp.tile([H2, cfg.T], BF16, name="sin_sb")
        wkv_sb = wkvp.tile([P, 2 * KT * cfg.HD], BF16, name="wkv_sb")
        nc.sync.dma_start(wkv_sb, wkv2)
        nc.sync.dma_start(cos_sb, cosT)
        nc.sync.dma_start(sin_sb, sinT)

        xsb = []
        xsb.append(xTp.tile([P, KT * BW], BF16, name="xsb0", tag="xT"))
        nc.sync.dma_start(xsb[0], xb2[0:P, :])

        # wq in 4 chunks of 7 ke-tiles for q-proj(b0) streaming
        wq_sb = wqp.tile([P, KT * HG], BF16, name="wq_sb")
        WQC = KT // 4 * HG  # 6272
        for c in range(4):
            nc.sync.dma_start(wq_sb[:, c * WQC:(c + 1) * WQC],
                              wq2[:, c * WQC:(c + 1) * WQC])

        xsb.append(xTp.tile([P, KT * BW], BF16, name="xsb1", tag="xT"))
        nc.sync.dma_start(xsb[1], xb2[P:2 * P, :])

        wo_sb = wop.tile([P, KT * EO], BF16, name="wo_sb")
        WOC = KT // 2 * EO
        for c in range(2):
            nc.sync.dma_start(wo_sb[:, c * WOC:(c + 1) * WOC],
                              wo2[:, c * WOC:(c + 1) * WOC])

        for b in range(2, NB):
            t = xTp.tile([P, KT * BW], BF16, name=f"xsb{b}", tag="xT")
            nc.sync.dma_start(t, xb2[b * P:(b + 1) * P, :])
            xsb.append(t)

        def xt(ke, b):
            return xsb[b][:, ke * BW:(ke + 1) * BW]

        def wkh(ke):
            return wkv_sb[:, ke * cfg.HD:(ke + 1) * cfg.HD]

        def wvh(ke):
            return wkv_sb[:, KT * cfg.HD + ke * cfg.HD:KT * cfg.HD + (ke + 1) * cfg.HD]

        def wqh(ke, h):
            return wq_sb[:, ke * HG + h * P:ke * HG + (h + 1) * P]

        def woh(kt, e):
            return wo_sb[:, kt * EO + e * P:kt * EO + (e + 1) * P]

        kT = kTp.tile([P, cfg.T], BF16, name="kT")
        vts = [vp.tile([P, cfg.HD], BF16, name=f"v{i}", tag="v")
               for i in range(2 * NB)]

        cc_in = [dramp.tile([P, GQ * BW], BF16, name=f"cc_in{b}")
                 for b in range(NB)]
        cc_out = [dramp.tile([4 * P, GQ * BW], BF16, name=f"cc_out{b}")
                  for b in range(NB)]

        def rope_drain(psum, dst, t0, w, ang0=None):
            """dst[:, t0:t0+w] = rope(psum); psum [128, w] f32, dst bf16.
            ang0: global token offset for the cos/sin tables (default t0)."""
            ang0 = t0 if ang0 is None else ang0
            c = cos_sb[:, ang0:ang0 + w]
            s = sin_sb[:, ang0:ang0 + w]
            p1 = psum[0:H2, :]
            p2 = psum[H2:P, :]
            t1 = rtp.tile([H2, BW], F32, name="t1", tag="rt1")[:, :w]
            t2 = rtp.tile([H2, BW], F32, name="t2", tag="rt2")[:, :w]
            nc.vector.tensor_mul(t1, p1, c)
            nc.vector.tensor_mul(t2, p2, s)
            nc.vector.tensor_sub(dst[0:H2, t0:t0 + w], t1, t2)
            nc.vector.tensor_mul(t1, p2, c)
            nc.vector.tensor_mul(t2, p1, s)
            nc.vector.tensor_add(dst[H2:P, t0:t0 + w], t1, t2)

        qT = {}  # (h, b) -> tile

        def kv_proj(b):
            t0 = b * BW
            # one bank: k in cols [0:256), v sub-tiles in [256:384) / [384:512)
            bank = pkvp.tile([P, 512], F32, name="pkv", tag="pkv")
            psk = bank[:, 0:BW]
            for ke in range(KT):
                nc.tensor.matmul(out=psk, lhsT=wkh(ke), rhs=xt(ke, b),
                                 start=(ke == 0), stop=(ke == KT - 1))
            rope_drain(psk, kT, t0, BW)
            for sub in range(2):
                pv = bank[:, BW + sub * P:BW + (sub + 1) * P]
                for ke in range(KT):
                    nc.tensor.matmul(
                        out=pv, lhsT=xt(ke, b)[:, sub * P:(sub + 1) * P],
                        rhs=wvh(ke),
                        start=(ke == 0), stop=(ke == KT - 1))
                nc.any.tensor_copy(vts[2 * b + sub], pv)

        def q_proj_heads(b, heads, ke_outer):
            if ke_outer:
                # one bank per head (<=2 heads) so the two interleaved
                # accumulation chains never share a PSUM bank
                assert len(heads) <= 2
                slot = {h: pqp.tile([P, 512], F32, name=f"pqo{h}",
                                    tag="pq")[:, 0:BW] for h in heads}
                for ke in range(KT):
                    for h in heads:
                        nc.tensor.matmul(
                            out=slot[h], lhsT=wqh(ke, h), rhs=xt(ke, b),
                            start=(ke == 0), stop=(ke == KT - 1))
                for h in heads:
                    q = qTp.tile([P, BW], BF16, name=f"qT{h}_{b}", tag="qT")
                    rope_drain(slot[h], q, 0, BW, ang0=b * BW)
                    qT[(h, b)] = q
            else:
                # sequential heads: pack pairs into one bank (chains are
                # temporally disjoint; rope of h overlaps proj of h+1)
                bank = None
                for i, h in enumerate(heads):
                    if i % 2 == 0:
                        bank = pqp.tile([P, 512], F32, name=f"pq{h}", tag="pq")
                    ps = bank[:, (i % 2) * BW:(i % 2 + 1) * BW]
                    for ke in range(KT):
                        nc.tensor.matmul(
                            out=ps, lhsT=wqh(ke, h), rhs=xt(ke, b),
                            start=(ke == 0), stop=(ke == KT - 1))
                    q = qTp.tile([P, BW], BF16, name=f"qT{h}_{b}", tag="qT")
                    rope_drain(ps, q, 0, BW, ang0=b * BW)
                    qT[(h, b)] = q

        def attn_head(b, h, qkv_blk):
            """Attention for head h, token block b -> qkv_blk[:, h*BW:...]."""
            nsi = 2 * (b + 1)
            plbank = plp.tile([P, 512], F32, name="plb", tag="pl")
            pts = []
            for si in range(nsi):
                c0 = 0 if si <= 2 * b else P
                cw = BW - c0
                pl = plbank[:, (si % 2) * BW:(si % 2) * BW + cw]
                diag = si >= 2 * b
                nc.tensor.matmul(
                    out=pl,
                    lhsT=kT[:, si * P:(si + 1) * P],
                    rhs=qT[(h, b)][:, c0:BW],
                    start=True, stop=not diag,
                )
                if diag:
                    # add NEG to masked (s_local > q_local) entries of the
                    # first 128 columns of this si's valid q range
                    nc.tensor.matmul(
                        out=pl[:, 0:P], lhsT=negd, rhs=rlow,
                        start=False, stop=True,
                    )
                pt = ptp.tile([P, BW], BF16, name="pt", tag="pt")[:, :cw]
                nc.scalar.activation(
                    pt, pl, mybir.ActivationFunctionType.Exp, scale=cfg.scale,
                )
                pts.append((pt, c0, cw))

            # shared bank: recb at [0:256), denominators sp at [0:1, 256:512)
            sprbank = sprp.tile([P, 512], F32, name="spr", tag="spr")
            sp = sprbank[0:1, BW:2 * BW]
            for si, (pt, c0, cw) in enumerate(pts):
                nc.tensor.matmul(
                    out=sp[:, c0:c0 + cw], lhsT=ones_bf, rhs=pt,
                    start=(si == 0), stop=(si == nsi - 1),
                )
            rec_bf = recp.tile([1, BW], BF16, name="rec", tag="rec")
            with nc.allow_low_precision("softmax reciprocal feeds bf16 matmul"):
                nc.vector.reciprocal(out=rec_bf, in_=sp)
            recb_ps = sprbank[:, 0:BW]
            nc.tensor.matmul(out=recb_ps, lhsT=ones1, rhs=rec_bf,
                             start=True, stop=True)
            # TensorTensor may read only one PSUM operand: drain recb to SBUF
            recb = recp.tile([P, BW], BF16, name="recb", tag="recb")
            nc.vector.tensor_copy(recb, recb_ps)

            pav = pavp.tile([P, BW], F32, name="pav", tag="pav")
            for si, (pt, c0, cw) in enumerate(pts):
                nc.tensor.matmul(
                    out=pav[:, c0:c0 + cw], lhsT=vts[si], rhs=pt,
                    start=(si == 0), stop=(si == nsi - 1),
                )
            nc.vector.tensor_mul(qkv_blk[:, h * BW:(h + 1) * BW], pav, recb)

        def allgather(b):
            nc.gpsimd.collective_compute(
                "AllGather",
                mybir.AluOpType.bypass,
                replica_groups=[[0, 1, 2, 3], [4, 5, 6, 7]],
                ins=[cc_in[b].opt()],
                outs=[cc_out[b].opt()],
            )

        def oproj(b):
            # one strided DMA: cc_out[b] [4*128, 7*256] -> [128, 28*256]
            qkh = qkhp.tile([P, NH * BW], BF16, name=f"qkh{b}", tag="qkh")
            nc.sync.dma_start(
                qkh, cc_out[b][:, :].rearrange("(r p) f -> p r f", p=P))
            osb = osbp.tile([P, cfg.ET * BW], F32, name=f"osb{b}", tag="osb")
            for e in range(cfg.ET):
                po = pop.tile([P, BW], F32, name="po", tag="po")
                for kt in range(NH):
                    nc.tensor.matmul(
                        out=po, lhsT=woh(kt, e), rhs=qkh[:, kt * BW:(kt + 1) * BW],
                        start=(kt == 0), stop=(kt == NH - 1),
                    )
                nc.any.tensor_copy(osb[:, e * BW:(e + 1) * BW], po)
            nc.sync.dma_start(
                oT_s[:, b * BW:(b + 1) * BW].rearrange("(e p) c -> p e c", p=P),
                osb)

        def attn_block(b, heads):
            qkv_blk = qkvbp.tile([P, GQ * BW], BF16, name=f"qkvb{b}", tag="qkvb")
            for h in heads:
                attn_head(b, h, qkv_blk)
            nc.sync.dma_start(cc_in[b], qkv_blk)
            allgather(b)

        # ================= pipeline =================
        kv_proj(0)
        q_proj_heads(0, [0, 1], ke_outer=True)
        q_proj_heads(0, [2, 3, 4, 5, 6], ke_outer=False)
        attn_block(0, list(range(GQ)))

        for b in range(1, NB):
            kv_proj(b)
            q_proj_heads(b, list(range(GQ)), ke_outer=False)
            attn_block(b, list(range(GQ)))
            oproj(b - 1)
        oproj(NB - 1)


# ======================= host side =======================

_NC_CACHE = {}


def _get_nc(cfg_key=None):
    if cfg_key not in _NC_CACHE:
        _NC_CACHE[cfg_key] = build_kernel(Cfg())
    return _NC_CACHE[cfg_key]


def _rope_tables(segment_ids, cur_ind, T, HD):
    valid = (np.asarray(segment_ids) != 0)
    pos = np.cumsum(valid, axis=-1) - 1 + int(cur_ind)  # [B, T]
    frac = 2.0 * np.arange(HD // 2, dtype=np.float64) / HD
    timescale = THETA ** frac
    ang = pos[..., None].astype(np.float64) / timescale  # [B, T, HD/2]
    cosT = np.transpose(np.cos(ang), (0, 2, 1)).astype(np.float32)  # [B, HD/2, T]
    sinT = np.transpose(np.sin(ang), (0, 2, 1)).astype(np.float32)
    return cosT, sinT


def _pack_km(w, P=128):
    """[K, M] -> [P, (K/P)*M]: tile ke on rows -> partition-major columns."""
    K, M = w.shape
    kt = K // P
    return np.ascontiguousarray(
        w.reshape(kt, P, M).transpose(1, 0, 2).reshape(P, kt * M))


def prepare_in_maps(inputs, cfg=None):
    import ml_dtypes
    bf16 = ml_dtypes.bfloat16
    cfg = cfg or Cfg()
    x = np.asarray(inputs["x"], dtype=np.float32)
    wq = np.asarray(inputs["wq"], dtype=np.float32).astype(bf16)
    wk = np.asarray(inputs["wk"], dtype=np.float32).astype(bf16)
    wv = np.asarray(inputs["wv"], dtype=np.float32).astype(bf16)
    wo = np.asarray(inputs["wo"], dtype=np.float32).astype(bf16)
    seg = np.asarray(inputs["segment_ids"])
    cur = int(np.asarray(inputs["cur_ind"]))

    B, T, EMB = x.shape
    assert (B, T, EMB) == (2, cfg.T, cfg.EMB)
    HG, HD, EO, KT = cfg.HG, cfg.HD, cfg.EO, cfg.KT
    cosT, sinT = _rope_tables(seg, cur, T, HD)
    xT = np.transpose(x, (0, 2, 1)).astype(bf16)  # [B, EMB, T]
    # xb2: [B, NB*P, KT*BW]; block b rows [b*P:(b+1)*P], x tile (ke,b) at
    # cols [ke*BW:(ke+1)*BW]
    xb2 = np.ascontiguousarray(
        xT.reshape(B, KT, P, NB, BW).transpose(0, 3, 2, 1, 4).reshape(
            B, NB * P, KT * BW))

    in_maps = []
    for c in range(8):
        b, j = c // 4, c % 4
        wkj = np.ascontiguousarray(wk[:, j * HD:(j + 1) * HD])
        wvj = np.ascontiguousarray(wv[:, j * HD:(j + 1) * HD])
        wkv2 = np.concatenate([_pack_km(wkj), _pack_km(wvj)], axis=1)
        in_maps.append({
            "xb2": xb2[b],
            "wq2": _pack_km(np.ascontiguousarray(wq[:, j * HG:(j + 1) * HG])),
            "wkv2": np.ascontiguousarray(wkv2),
            "wo2": _pack_km(np.ascontiguousarray(wo[:, j * EO:(j + 1) * EO])),
            "cosT": np.ascontiguousarray(cosT[b]).astype(bf16),
            "sinT": np.ascontiguousarray(sinT[b]).astype(bf16),
        })
    return in_maps


def assemble_out(results, cfg=None):
    cfg = cfg or Cfg()
    out = np.empty((2, cfg.T, cfg.EMB), np.float32)
    for c in range(8):
        b, j = c // 4, c % 4
        out[b, :, j * cfg.EO:(j + 1) * cfg.EO] = results[c]["oT_s"].T
    return out


def kernel(**inputs):
    cfg = Cfg()
    in_maps = prepare_in_maps(inputs, cfg)
    nc = _get_nc()
    res = run_bass_kernel_spmd(nc, in_maps, core_ids=list(range(8)))
    return assemble_out(res.results, cfg)


# revision 18
# speedup vs baseline: 1.7470x; 1.0075x over previous
"""Distributed Trainium2 Bass kernel for GQA attention (nn_Attention_27814208209106).

Sharding: 8 cores = 2 batches x 4 KV-head groups (7 q-heads + 1 kv head each).
v3: 4x256-token block pipeline with batched DMAs (host pre-packs every
weight/activation into partition-major monoliths so each logical transfer is
ONE descriptor: ~25 DMA issues total vs ~260 in v2 -- the SP sequencer at
~565ns/issue was gating the input stream). Small AllGathers (1.83MB out, one
per block) overlap compute; reciprocal broadcast + causal diag mask run on
the PE; o-proj in emb-partitioned layout.

All matmuls bf16 with f32 PSUM accumulation. PSUM banks are packed with
region-disjoint accumulation chains (8-bank budget).
"""

import numpy as np

import concourse.bass as bass
import concourse.mybir as mybir
import concourse.tile as tile
from concourse import bacc
from concourse.bass_utils import run_bass_kernel_spmd

P = 128
BW = 256           # token block width
NB = 4             # number of token blocks
THETA = 1000000.0
NEG = -30000.0

F32 = mybir.dt.float32
BF16 = mybir.dt.bfloat16


class Cfg:
    def __init__(self, T=1024, EMB=3584, NH=28, KVH=4, HD=128):
        self.T, self.EMB, self.NH, self.KVH, self.HD = T, EMB, NH, KVH, HD
        self.GQ = NH // KVH          # q heads per kv head (7)
        self.HG = self.GQ * HD       # per-core q width (896)
        self.NHD = NH * HD           # full qkv width (3584)
        self.EO = EMB // 4           # o-proj output slice per core (896)
        self.KT = EMB // P           # contraction tiles (28)
        self.ET = self.EO // P       # o-proj emb tiles (7)
        self.scale = HD ** -0.5


def build_kernel(cfg: Cfg):
    nc = bacc.Bacc(
        "TRN2",
        target_bir_lowering=False,
        debug=False,
        enable_asserts=False,
        num_devices=8,
    )

    KT, HG, EO = cfg.KT, cfg.HG, cfg.EO
    xb2 = nc.dram_tensor("xb2", [NB * P, KT * BW], BF16, kind="ExternalInput").ap()
    wq2 = nc.dram_tensor("wq2", [P, KT * HG], BF16, kind="ExternalInput").ap()
    wkv2 = nc.dram_tensor("wkv2", [P, 2 * KT * cfg.HD], BF16, kind="ExternalInput").ap()
    wo2 = nc.dram_tensor("wo2", [P, KT * EO], BF16, kind="ExternalInput").ap()
    cosT = nc.dram_tensor("cosT", [cfg.HD // 2, cfg.T], BF16, kind="ExternalInput").ap()
    sinT = nc.dram_tensor("sinT", [cfg.HD // 2, cfg.T], BF16, kind="ExternalInput").ap()
    oT_s = nc.dram_tensor("oT_s", [EO, cfg.T], F32, kind="ExternalOutput").ap()

    with tile.TileContext(nc) as tc:
        _body(tc, cfg, xb2, wq2, wkv2, wo2, cosT, sinT, oT_s)

    nc.compile()
    return nc


def _body(tc, cfg, xb2, wq2, wkv2, wo2, cosT, sinT, oT_s):
    nc = tc.nc
    H2 = cfg.HD // 2
    KT, HG, EO, NH, GQ = cfg.KT, cfg.HG, cfg.EO, cfg.NH, cfg.GQ

    from contextlib import ExitStack
    with ExitStack() as _st:
        ec = _st.enter_context
        constp = ec(tc.tile_pool(name="const", bufs=1))
        csp = ec(tc.tile_pool(name="cs", bufs=1))
        xTp = ec(tc.tile_pool(name="xT", bufs=2))
        wqp = ec(tc.tile_pool(name="wq", bufs=1))
        wkvp = ec(tc.tile_pool(name="wkv", bufs=1))
        wop = ec(tc.tile_pool(name="wo", bufs=1))
        kTp = ec(tc.tile_pool(name="kT", bufs=1))
        vp = ec(tc.tile_pool(name="vv", bufs=2 * NB))
        qTp = ec(tc.tile_pool(name="qT", bufs=2 * cfg.GQ))
        ptp = ec(tc.tile_pool(name="pt", bufs=8))
        recp = ec(tc.tile_pool(name="rec", bufs=2))
        qkvbp = ec(tc.tile_pool(name="qkvb", bufs=1))
        qkhp = ec(tc.tile_pool(name="qkh", bufs=2))
        osbp = ec(tc.tile_pool(name="osb", bufs=1))
        rtp = ec(tc.tile_pool(name="rtmp", bufs=2))
        dramp = ec(tc.tile_pool(name="dram", bufs=1, space="DRAM"))
        # PSUM: 8 banks x 2KB/partition, region-packed accumulation chains.
        pkvp = ec(tc.tile_pool(name="pkv", bufs=1, space="PSUM"))  # 1 bank
        pqp = ec(tc.tile_pool(name="pq", bufs=1, space="PSUM"))    # 1 bank
        plp = ec(tc.tile_pool(name="pl", bufs=2, space="PSUM"))    # 2 banks
        sprp = ec(tc.tile_pool(name="spr", bufs=1, space="PSUM"))  # 1 bank
        pavp = ec(tc.tile_pool(name="pav", bufs=1, space="PSUM"))  # 1 bank
        pop = ec(tc.tile_pool(name="po", bufs=2, space="PSUM"))    # 2 banks

        # ---- constants ----
        ident = constp.tile([P, P], BF16, name="ident")
        nc.gpsimd.memset(ident, 0.0)
        nc.gpsimd.affine_select(
            out=ident, in_=ident, compare_op=mybir.AluOpType.not_equal,
            fill=1.0, base=0, pattern=[[-1, P]], channel_multiplier=1,
        )
        # negd = diag(NEG)
        negd = constp.tile([P, P], BF16, name="negd")
        nc.gpsimd.memset(negd, 0.0)
        nc.gpsimd.affine_select(
            out=negd, in_=negd, compare_op=mybir.AluOpType.not_equal,
            fill=NEG, base=0, pattern=[[-1, P]], channel_multiplier=1,
        )
        # rlow[s, q] = 1 iff s > q  (strict lower triangle)
        rlow = constp.tile([P, P], BF16, name="rlow")
        nc.gpsimd.memset(rlow, 0.0)
        nc.gpsimd.affine_select(
            out=rlow, in_=rlow, compare_op=mybir.AluOpType.is_ge,
            fill=1.0, base=0, pattern=[[1, P]], channel_multiplier=-1,
        )
        ones_bf = constp.tile([P, 1], BF16, name="ones_bf")
        nc.vector.memset(ones_bf, 1.0)
        ones1 = constp.tile([1, P], BF16, name="ones1")
        nc.vector.memset(ones1, 1.0)
        wrm = constp.tile([P, BW], BF16, name="wrm")
        nc.vector.memset(wrm, 0.0)

        def warm_burst(n):
            pw = pop.tile([P, BW], F32, name="pwb", tag="po")
            for _ in range(n):
                nc.tensor.matmul(out=pw, lhsT=ident, rhs=wrm, start=True, stop=True)

        # PE warmup while first DMAs stream in
        warm_burst(40)

        # ---- batched DMA staging ----
        cos_sb = csp.tile([H2, cfg.T], BF16, name="cos_sb")
        sin_sb = csp.tile([H2, cfg.T], BF16, name="sin_sb")
        wkv_sb = wkvp.tile([P, 2 * KT * cfg.HD], BF16, name="wkv_sb")
        nc.sync.dma_start(wkv_sb, wkv2)
        nc.sync.dma_start(cos_sb, cosT)
        nc.sync.dma_start(sin_sb, sinT)

        xsb = []
        xsb.append(xTp.tile([P, KT * BW], BF16, name="xsb0", tag="xT"))
        nc.sync.dma_start(xsb[0], xb2[0:P, :])

        # wq in 4 chunks of 7 ke-tiles for q-proj(b0) streaming
        wq_sb = wqp.tile([P, KT * HG], BF16, name="wq_sb")
        WQC = KT // 4 * HG  # 6272
        for c in range(4):
            nc.sync.dma_start(wq_sb[:, c * WQC:(c + 1) * WQC],
                              wq2[:, c * WQC:(c + 1) * WQC])

        xsb.append(xTp.tile([P, KT * BW], BF16, name="xsb1", tag="xT"))
        nc.sync.dma_start(xsb[1], xb2[P:2 * P, :])

        wo_sb = wop.tile([P, KT * EO], BF16, name="wo_sb")
        WOC = KT // 2 * EO
        for c in range(2):
            nc.sync.dma_start(wo_sb[:, c * WOC:(c + 1) * WOC],
                              wo2[:, c * WOC:(c + 1) * WOC])

        for b in range(2, NB):
            t = xTp.tile([P, KT * BW], BF16, name=f"xsb{b}", tag="xT")
            nc.sync.dma_start(t, xb2[b * P:(b + 1) * P, :])
            xsb.append(t)

        def xt(ke, b):
            return xsb[b][:, ke * BW:(ke + 1) * BW]

        def wkh(ke):
            return wkv_sb[:, ke * cfg.HD:(ke + 1) * cfg.HD]

        def wvh(ke):
            return wkv_sb[:, KT * cfg.HD + ke * cfg.HD:KT * cfg.HD + (ke + 1) * cfg.HD]

        def wqh(ke, h):
            return wq_sb[:, ke * HG + h * P:ke * HG + (h + 1) * P]

        def woh(kt, e):
            return wo_sb[:, kt * EO + e * P:kt * EO + (e + 1) * P]

        kT = kTp.tile([P, cfg.T], BF16, name="kT")
        vts = [vp.tile([P, cfg.HD], BF16, name=f"v{i}", tag="v")
               for i in range(2 * NB)]

        cc_in = [dramp.tile([P, GQ * BW], BF16, name=f"cc_in{b}")
                 for b in range(NB)]
        cc_out = [dramp.tile([4 * P, GQ * BW], BF16, name=f"cc_out{b}")
                  for b in range(NB)]

        def rope_drain(psum, dst, t0, w, ang0=None):
            """dst[:, t0:t0+w] = rope(psum); psum [128, w] f32, dst bf16.
            ang0: global token offset for the cos/sin tables (default t0)."""
            ang0 = t0 if ang0 is None else ang0
            c = cos_sb[:, ang0:ang0 + w]
            s = sin_sb[:, ang0:ang0 + w]
            p1 = psum[0:H2, :]
            p2 = psum[H2:P, :]
            t1 = rtp.tile([H2, BW], F32, name="t1", tag="rt1")[:, :w]
            t2 = rtp.tile([H2, BW], F32, name="t2", tag="rt2")[:, :w]
            nc.vector.tensor_mul(t1, p1, c)
            nc.vector.tensor_mul(t2, p2, s)
            nc.vector.tensor_sub(dst[0:H2, t0:t0 + w], t1, t2)
            nc.vector.tensor_mul(t1, p2, c)
            nc.vector.tensor_mul(t2, p1, s)
            nc.vector.tensor_add(dst[H2:P, t0:t0 + w], t1, t2)

        qT = {}  # (h, b) -> tile

        def kv_proj(b):
            t0 = b * BW
            # one bank: k in cols [0:256), v sub-tiles in [256:384) / [384:512)
            bank = pkvp.tile([P, 512], F32, name="pkv", tag="pkv")
            psk = bank[:, 0:BW]
            for ke in range(KT):
                nc.tensor.matmul(out=psk, lhsT=wkh(ke), rhs=xt(ke, b),
                                 start=(ke == 0), stop=(ke == KT - 1))
            rope_drain(psk, kT, t0, BW)
            for sub in range(2):
                pv = bank[:, BW + sub * P:BW + (sub + 1) * P]
                for ke in range(KT):
                    nc.tensor.matmul(
                        out=pv, lhsT=xt(ke, b)[:, sub * P:(sub + 1) * P],
                        rhs=wvh(ke),
                        start=(ke == 0), stop=(ke == KT - 1))
                nc.scalar.copy(vts[2 * b + sub], pv)

        def q_proj_heads(b, heads, ke_outer):
            if ke_outer:
                # one bank per head (<=2 heads) so the two interleaved
                # accumulation chains never share a PSUM bank
                assert len(heads) <= 2
                slot = {h: pqp.tile([P, 512], F32, name=f"pqo{h}",
                                    tag="pq")[:, 0:BW] for h in heads}
                for ke in range(KT):
                    for h in heads:
                        nc.tensor.matmul(
                            out=slot[h], lhsT=wqh(ke, h), rhs=xt(ke, b),
                            start=(ke == 0), stop=(ke == KT - 1))
                for h in heads:
                    q = qTp.tile([P, BW], BF16, name=f"qT{h}_{b}", tag="qT")
                    rope_drain(slot[h], q, 0, BW, ang0=b * BW)
                    qT[(h, b)] = q
            else:
                # sequential heads: pack pairs into one bank (chains are
                # temporally disjoint; rope of h overlaps proj of h+1)
                bank = None
                for i, h in enumerate(heads):
                    if i % 2 == 0:
                        bank = pqp.tile([P, 512], F32, name=f"pq{h}", tag="pq")
                    ps = bank[:, (i % 2) * BW:(i % 2 + 1) * BW]
                    for ke in range(KT):
                        nc.tensor.matmul(
                            out=ps, lhsT=wqh(ke, h), rhs=xt(ke, b),
                            start=(ke == 0), stop=(ke == KT - 1))
                    q = qTp.tile([P, BW], BF16, name=f"qT{h}_{b}", tag="qT")
                    rope_drain(ps, q, 0, BW, ang0=b * BW)
                    qT[(h, b)] = q

        def attn_head(b, h, qkv_blk):
            """Attention for head h, token block b -> qkv_blk[:, h*BW:...]."""
            nsi = 2 * (b + 1)
            plbank = plp.tile([P, 512], F32, name="plb", tag="pl")
            pts = []
            for si in range(nsi):
                c0 = 0 if si <= 2 * b else P
                cw = BW - c0
                pl = plbank[:, (si % 2) * BW:(si % 2) * BW + cw]
                diag = si >= 2 * b
                nc.tensor.matmul(
                    out=pl,
                    lhsT=kT[:, si * P:(si + 1) * P],
                    rhs=qT[(h, b)][:, c0:BW],
                    start=True, stop=not diag,
                )
                if diag:
                    # add NEG to masked (s_local > q_local) entries of the
                    # first 128 columns of this si's valid q range
                    nc.tensor.matmul(
                        out=pl[:, 0:P], lhsT=negd, rhs=rlow,
                        start=False, stop=True,
                    )
                pt = ptp.tile([P, BW], BF16, name="pt", tag="pt")[:, :cw]
                nc.scalar.activation(
                    pt, pl, mybir.ActivationFunctionType.Exp, scale=cfg.scale,
                )
                pts.append((pt, c0, cw))

            # shared bank: recb at [0:256), denominators sp at [0:1, 256:512)
            sprbank = sprp.tile([P, 512], F32, name="spr", tag="spr")
            sp = sprbank[0:1, BW:2 * BW]
            for si, (pt, c0, cw) in enumerate(pts):
                nc.tensor.matmul(
                    out=sp[:, c0:c0 + cw], lhsT=ones_bf, rhs=pt,
                    start=(si == 0), stop=(si == nsi - 1),
                )
            rec_bf = recp.tile([1, BW], BF16, name="rec", tag="rec")
            with nc.allow_low_precision("softmax reciprocal feeds bf16 matmul"):
                nc.vector.reciprocal(out=rec_bf, in_=sp)
            recb_ps = sprbank[:, 0:BW]
            nc.tensor.matmul(out=recb_ps, lhsT=ones1, rhs=rec_bf,
                             start=True, stop=True)
            # TensorTensor may read only one PSUM operand: drain recb to SBUF
            recb = recp.tile([P, BW], BF16, name="recb", tag="recb")
            nc.vector.tensor_copy(recb, recb_ps)

            pav = pavp.tile([P, BW], F32, name="pav", tag="pav")
            for si, (pt, c0, cw) in enumerate(pts):
                nc.tensor.matmul(
                    out=pav[:, c0:c0 + cw], lhsT=vts[si], rhs=pt,
                    start=(si == 0), stop=(si == nsi - 1),
                )
            nc.vector.tensor_mul(qkv_blk[:, h * BW:(h + 1) * BW], pav, recb)

        def allgather(b):
            nc.gpsimd.collective_compute(
                "AllGather",
                mybir.AluOpType.bypass,
                replica_groups=[[0, 1, 2, 3], [4, 5, 6, 7]],
                ins=[cc_in[b].opt()],
                outs=[cc_out[b].opt()],
            )

        def oproj(b, qkh):
            osb = osbp.tile([P, cfg.ET * BW], F32, name=f"osb{b}", tag="osb")
            for e in range(cfg.ET):
                po = pop.tile([P, BW], F32, name="po", tag="po")
                for kt in range(NH):
                    nc.tensor.matmul(
                        out=po, lhsT=woh(kt, e), rhs=qkh[:, kt * BW:(kt + 1) * BW],
                        start=(kt == 0), stop=(kt == NH - 1),
                    )
                nc.scalar.copy(osb[:, e * BW:(e + 1) * BW], po)
            nc.sync.dma_start(
                oT_s[:, b * BW:(b + 1) * BW].rearrange("(e p) c -> p e c", p=P),
                osb)

        def attn_block(b, heads):
            qkv_blk = qkvbp.tile([P, GQ * BW], BF16, name=f"qkvb{b}", tag="qkvb")
            for h in heads:
                attn_head(b, h, qkv_blk)
            nc.sync.dma_start(cc_in[b], qkv_blk)
            allgather(b)
            # prefetch gathered qkv^T: one strided DMA [4*128, 7*256] -> [128, 28*256]
            qkh = qkhp.tile([P, NH * BW], BF16, name=f"qkh{b}", tag="qkh")
            nc.sync.dma_start(
                qkh, cc_out[b][:, :].rearrange("(r p) f -> p r f", p=P))
            return qkh

        # ================= pipeline =================
        qkhs = []
        for b in range(NB):
            kv_proj(b)
            q_proj_heads(b, list(range(GQ)), ke_outer=False)
            qkhs.append(attn_block(b, list(range(GQ))))
            if b >= 2:
                oproj(b - 2, qkhs[b - 2])
        oproj(NB - 2, qkhs[NB - 2])
        oproj(NB - 1, qkhs[NB - 1])


# ======================= host side =======================

_NC_CACHE = {}


def _get_nc(cfg_key=None):
    if cfg_key not in _NC_CACHE:
        _NC_CACHE[cfg_key] = build_kernel(Cfg())
    return _NC_CACHE[cfg_key]


def _rope_tables(segment_ids, cur_ind, T, HD):
    valid = (np.asarray(segment_ids) != 0)
    pos = np.cumsum(valid, axis=-1) - 1 + int(cur_ind)  # [B, T]
    frac = 2.0 * np.arange(HD // 2, dtype=np.float64) / HD
    timescale = THETA ** frac
    ang = pos[..., None].astype(np.float64) / timescale  # [B, T, HD/2]
    cosT = np.transpose(np.cos(ang), (0, 2, 1)).astype(np.float32)  # [B, HD/2, T]
    sinT = np.transpose(np.sin(ang), (0, 2, 1)).astype(np.float32)
    return cosT, sinT


def _pack_km(w, P=128):
    """[K, M] -> [P, (K/P)*M]: tile ke on rows -> partition-major columns."""
    K, M = w.shape
    kt = K // P
    return np.ascontiguousarray(
        w.reshape(kt, P, M).transpose(1, 0, 2).reshape(P, kt * M))


def prepare_in_maps(inputs, cfg=None):
    import ml_dtypes
    bf16 = ml_dtypes.bfloat16
    cfg = cfg or Cfg()
    x = np.asarray(inputs["x"], dtype=np.float32)
    wq = np.asarray(inputs["wq"], dtype=np.float32).astype(bf16)
    wk = np.asarray(inputs["wk"], dtype=np.float32).astype(bf16)
    wv = np.asarray(inputs["wv"], dtype=np.float32).astype(bf16)
    wo = np.asarray(inputs["wo"], dtype=np.float32).astype(bf16)
    seg = np.asarray(inputs["segment_ids"])
    cur = int(np.asarray(inputs["cur_ind"]))

    B, T, EMB = x.shape
    assert (B, T, EMB) == (2, cfg.T, cfg.EMB)
    HG, HD, EO, KT = cfg.HG, cfg.HD, cfg.EO, cfg.KT
    cosT, sinT = _rope_tables(seg, cur, T, HD)
    xT = np.transpose(x, (0, 2, 1)).astype(bf16)  # [B, EMB, T]
    # xb2: [B, NB*P, KT*BW]; block b rows [b*P:(b+1)*P], x tile (ke,b) at
    # cols [ke*BW:(ke+1)*BW]
    xb2 = np.ascontiguousarray(
        xT.reshape(B, KT, P, NB, BW).transpose(0, 3, 2, 1, 4).reshape(
            B, NB * P, KT * BW))

    in_maps = []
    for c in range(8):
        b, j = c // 4, c % 4
        wkj = np.ascontiguousarray(wk[:, j * HD:(j + 1) * HD])
        wvj = np.ascontiguousarray(wv[:, j * HD:(j + 1) * HD])
        wkv2 = np.concatenate([_pack_km(wkj), _pack_km(wvj)], axis=1)
        in_maps.append({
            "xb2": xb2[b],
            "wq2": _pack_km(np.ascontiguousarray(wq[:, j * HG:(j + 1) * HG])),
            "wkv2": np.ascontiguousarray(wkv2),
            "wo2": _pack_km(np.ascontiguousarray(wo[:, j * EO:(j + 1) * EO])),
            "cosT": np.ascontiguousarray(cosT[b]).astype(bf16),
            "sinT": np.ascontiguousarray(sinT[b]).astype(bf16),
        })
    return in_maps


def assemble_out(results, cfg=None):
    cfg = cfg or Cfg()
    out = np.empty((2, cfg.T, cfg.EMB), np.float32)
    for c in range(8):
        b, j = c // 4, c % 4
        out[b, :, j * cfg.EO:(j + 1) * cfg.EO] = results[c]["oT_s"].T
    return out


def kernel(**inputs):
    cfg = Cfg()
    in_maps = prepare_in_maps(inputs, cfg)
    nc = _get_nc()
    res = run_bass_kernel_spmd(nc, in_maps, core_ids=list(range(8)))
    return assemble_out(res.results, cfg)
